# revision 26
# baseline (speedup 1.0000x reference)
"""Trainium2 Bass kernel for nn_Net_74552042324489.

Data-parallel over batch n=8 across 8 NeuronCores (1 sample/core).
Per-core pipeline:
  cam = fc8_w @ _4            -> norm/suppress -> camT5 = [bg|fg supp|ones]^T
  x2r = bilinear(x2,112->56)   (stride-2 DVE trick, align_corners)
  f8_3 = relu(f83_w @ x2r)
  f8_4 = relu(f84_w @ deep3)
  x_s = bilinear(x,448->56)    (dense resize-matrix matmuls on PE)
  f = [f8_4; f8_3; x_s]        (channel-permuted; qk weights permuted to match)
  q,k = Wqk @ f
  Attention: S blocked [h=128p, k free]; exp on ScalarE (no max-sub needed:
  |S|<~30); second matmul lhsT=[camT|ones] fuses numerator + softmax denom;
  divide at the end.  Output [4, 3136] per core.
"""

import os
import sys

sys.path.insert(0, "/opt/trn_rl_repo")

from contextlib import ExitStack

import numpy as np

import concourse.bass as bass
import concourse.tile as tile
from concourse import bacc, mybir
from concourse.bass_utils import run_bass_kernel_spmd
from concourse.masks import make_identity

F32 = mybir.dt.float32
BF16 = mybir.dt.bfloat16
AF = mybir.ActivationFunctionType
ALU = mybir.AluOpType

HW = 3136  # 56*56
N_CORES = 8

_CACHE = {}


def _resize_mat(h_in: int, h_out: int) -> np.ndarray:
    """Dense [h_in, h_out] bilinear align_corners=True resize matrix."""
    ys = np.linspace(0.0, h_in - 1.0, h_out).astype(np.float32)
    y0 = np.floor(ys).astype(np.int64)
    y1 = np.minimum(y0 + 1, h_in - 1)
    w = (ys - y0).astype(np.float32)
    R = np.zeros((h_in, h_out), np.float32)
    for i in range(h_out):
        R[y0[i], i] += 1.0 - w[i]
        R[y1[i], i] += w[i]
    return R


def _resize_coeffs_112() -> tuple[np.ndarray, np.ndarray]:
    """Per-output-col (0..54) weights for the stride-2 112->56 resize."""
    ys = np.linspace(0.0, 111.0, 56).astype(np.float32)
    y0 = np.floor(ys).astype(np.int64)
    w = (ys - y0).astype(np.float32)
    # structural property (verified): y0[i] == 2i for i < 55; y0[55] == 111
    a = (1.0 - w).astype(np.float32)  # weight of in[2i]
    b = w.astype(np.float32)          # weight of in[2i+1]
    return a, b


def _build_program():
    nc = bacc.Bacc(
        "TRN2", target_bir_lowering=False, debug=False, num_devices=N_CORES
    )

    # ---- DRAM I/O ----
    d_x4 = nc.dram_tensor("x4", [512, HW], F32, kind="ExternalInput")
    d_deep3 = nc.dram_tensor("deep3", [320, HW], F32, kind="ExternalInput")
    d_x2 = nc.dram_tensor("x2", [128, 112 * 112], F32, kind="ExternalInput")
    d_x = nc.dram_tensor("x", [3, 448, 448], BF16, kind="ExternalInput")
    d_fc8T = nc.dram_tensor("fc8T", [512, 4], F32, kind="ExternalInput")
    d_f83T = nc.dram_tensor("f83T", [128, 64], F32, kind="ExternalInput")
    d_f84T = nc.dram_tensor("f84T", [320, 128], F32, kind="ExternalInput")
    d_qkA = nc.dram_tensor("qkA", [128, 384], F32, kind="ExternalInput")
    d_qkB = nc.dram_tensor("qkB", [67, 384], F32, kind="ExternalInput")
    d_a112 = nc.dram_tensor("a112", [128, 56], F32, kind="ExternalInput")
    d_b112 = nc.dram_tensor("b112", [128, 56], F32, kind="ExternalInput")
    d_rh = nc.dram_tensor("rh448", [448, 56], BF16, kind="ExternalInput")
    d_rw = nc.dram_tensor("rw448", [448, 56], BF16, kind="ExternalInput")
    d_out = nc.dram_tensor("out", [4, HW], F32, kind="ExternalOutput")

    EPS = 1e-05

    with tile.TileContext(nc) as tc, ExitStack() as top:
        wpool = top.enter_context(tc.tile_pool(name="wpool", bufs=1))
        persist = top.enter_context(tc.tile_pool(name="persist", bufs=1))
        small = top.enter_context(tc.tile_pool(name="small", bufs=2))

        # ---- weights to SBUF ----
        fc8T = wpool.tile([128, 4, 4], F32, tag="fc8T")
        nc.sync.dma_start(fc8T[:], d_fc8T.ap().rearrange("(k p) o -> p k o", p=128))
        f83T = wpool.tile([128, 64], F32, tag="f83T")
        nc.sync.dma_start(f83T[:], d_f83T.ap())
        f84T_0 = wpool.tile([128, 128], F32, tag="f84T0")
        nc.sync.dma_start(f84T_0[:], d_f84T.ap()[0:128, :])
        f84T_1 = wpool.tile([128, 128], F32, tag="f84T1")
        nc.sync.dma_start(f84T_1[:], d_f84T.ap()[128:256, :])
        f84T_2 = wpool.tile([64, 128], F32, tag="f84T2")
        nc.sync.dma_start(f84T_2[:], d_f84T.ap()[256:320, :])
        qkA = wpool.tile([128, 384], F32, tag="qkA")
        nc.sync.dma_start(qkA[:], d_qkA.ap())
        qkB = wpool.tile([67, 384], F32, tag="qkB")
        nc.sync.dma_start(qkB[:], d_qkB.ap())
        a112 = wpool.tile([128, 56], F32, tag="a112")
        nc.sync.dma_start(a112[:], d_a112.ap())
        b112 = wpool.tile([128, 56], F32, tag="b112")
        nc.sync.dma_start(b112[:], d_b112.ap())
        rh = wpool.tile([112, 4, 56], BF16, tag="rh")
        nc.sync.dma_start(rh[:], d_rh.ap().rearrange("(k p) o -> p k o", p=112))
        rw = wpool.tile([112, 4, 56], BF16, tag="rw")
        nc.sync.dma_start(rw[:], d_rw.ap().rearrange("(k p) o -> p k o", p=112))
        ident = wpool.tile([128, 128], F32, tag="ident")
        make_identity(nc, ident[:])

        camT5 = persist.tile([128, 125], BF16, tag="camT5")  # 25 h-blocks x 5
        f_a = persist.tile([128, HW], F32, tag="f_a")  # = f8_4
        f_b = persist.tile([67, HW], F32, tag="f_b")  # = [f8_3(64); x_s(3)]
        qA = persist.tile([128, HW], BF16, tag="qA")
        qB = persist.tile([64, HW], BF16, tag="qB")
        kA = persist.tile([128, HW], BF16, tag="kA")
        kB = persist.tile([64, HW], BF16, tag="kB")
        out_sb = persist.tile([4, HW], F32, tag="out_sb")

        # h-block partition sizes: 24 x 128 + 1 x 64
        HBLK = [(i * 128, 128) for i in range(24)] + [(3072, 64)]
        # free-dim 512 chunks of 3136: 6 x 512 + 64
        NCH = [(i * 512, 512) for i in range(6)] + [(3072, 64)]

        # ================= P1: cam, norm, camT5 =================
        with tc.tile_pool(name="p1s", bufs=4) as p1s, \
             tc.tile_pool(name="p1p", bufs=4, space=bass.MemorySpace.PSUM) as p1p, \
             tc.tile_pool(name="p1sb", bufs=1) as p1sb:
            cam = p1sb.tile([4, HW], F32, tag="cam")
            for no, nl in NCH:
                cp = p1p.tile([4, 512], F32, tag="campsum")
                for ck in range(4):
                    st = p1s.tile([128, 512], F32, tag="x4st")
                    nc.sync.dma_start(
                        st[:, 0:nl], d_x4.ap()[128 * ck:128 * (ck + 1), no:no + nl]
                    )
                    nc.tensor.matmul(
                        cp[:, 0:nl], fc8T[:, ck, :], st[:, 0:nl],
                        start=(ck == 0), stop=(ck == 3),
                    )
                nc.vector.tensor_copy(cam[:, no:no + nl], cp[:, 0:nl])

            mn = small.tile([4, 1], F32, tag="mn")
            mx = small.tile([4, 1], F32, tag="mx")
            nc.vector.tensor_reduce(mn[:], cam[:], axis=mybir.AxisListType.X, op=ALU.min)
            nc.vector.tensor_reduce(mx[:], cam[:], axis=mybir.AxisListType.X, op=ALU.max)
            rng = small.tile([4, 1], F32, tag="rng")
            nc.vector.tensor_tensor(rng[:], mx[:], mn[:], op=ALU.subtract)
            nc.vector.tensor_scalar_add(rng[:], rng[:], EPS)
            rs = small.tile([4, 1], F32, tag="rs")
            nc.vector.reciprocal(rs[:], rng[:])
            norm = p1sb.tile([4, HW], F32, tag="norm")
            nc.vector.tensor_scalar(
                norm[:], cam[:], mn[:], rs[:], op0=ALU.subtract, op1=ALU.mult
            )

            # ones column (every 5th col of camT5)
            nc.vector.memset(camT5[:].rearrange("p (b f) -> p b f", f=5)[:, :, 4], 1.0)
            for bi, (ho, hl) in enumerate(HBLK):
                tp = p1p.tile([128, 4], F32, tag="tpsum")
                nc.tensor.transpose(tp[0:hl, :], norm[:, ho:ho + hl], ident[0:4, 0:4])
                fm = small.tile([128, 1], F32, tag="fm")
                nc.vector.tensor_reduce(
                    fm[0:hl], tp[0:hl, 1:4], axis=mybir.AxisListType.X, op=ALU.max
                )
                # bg = 1 - fgmax
                nc.vector.tensor_scalar(
                    camT5[0:hl, bi * 5:bi * 5 + 1], fm[0:hl], -1.0, 1.0,
                    op0=ALU.mult, op1=ALU.add,
                )
                msk = small.tile([128, 3], F32, tag="msk")
                nc.vector.tensor_scalar(
                    msk[0:hl], tp[0:hl, 1:4], fm[0:hl], None, op0=ALU.is_ge
                )
                nc.vector.tensor_tensor(
                    camT5[0:hl, bi * 5 + 1:bi * 5 + 4], tp[0:hl, 1:4], msk[0:hl],
                    op=ALU.mult,
                )

        # ================= P2: x2 -> x2r (stride-2 bilinear) =================
        with tc.tile_pool(name="p2w", bufs=1) as p2w, \
             tc.tile_pool(name="p2s", bufs=2) as p2s, \
             tc.tile_pool(name="p2r", bufs=1) as p2r:
            x2w = p2w.tile([128, 112 * 56], F32, tag="x2w")  # after W-resize
            x2wv = x2w[:].rearrange("p (h w) -> p h w", h=112)
            HC = 14  # h rows per W-stage chunk
            for hc in range(112 // HC):
                st = p2s.tile([128, HC * 112], F32, tag="x2st")
                nc.sync.dma_start(
                    st[:], d_x2.ap()[:, hc * HC * 112:(hc + 1) * HC * 112]
                )
                sv = st[:].rearrange("p (h w) -> p h w", h=HC)
                dst = x2wv[:, hc * HC:(hc + 1) * HC, :]
                even = sv[:, :, 0:110:2]   # 55 taps
                odd = sv[:, :, 1:111:2]
                abc = a112[:, 0:55].unsqueeze(1).broadcast_to([128, HC, 55])
                bbc = b112[:, 0:55].unsqueeze(1).broadcast_to([128, HC, 55])
                t1 = p2s.tile([128, HC, 55], F32, tag="t1")
                nc.vector.tensor_tensor(t1[:], even, abc, op=ALU.mult)
                t2 = p2s.tile([128, HC, 55], F32, tag="t2")
                nc.vector.tensor_tensor(t2[:], odd, bbc, op=ALU.mult)
                nc.vector.tensor_tensor(dst[:, :, 0:55], t1[:], t2[:], op=ALU.add)
                nc.vector.tensor_copy(dst[:, :, 55:56], sv[:, :, 111:112])

            x2r = p2r.tile([128, HW], F32, tag="x2r")
            x2rv = x2r[:].rearrange("p (h w) -> p h w", h=56)
            for jc, jl in ((0, 28), (28, 27)):
                everow = x2wv[:, 2 * jc:2 * (jc + jl) - 1:2, :]
                oddrow = x2wv[:, 2 * jc + 1:2 * (jc + jl):2, :]
                arow = a112[:, jc:jc + jl].unsqueeze(2).broadcast_to([128, jl, 56])
                brow = b112[:, jc:jc + jl].unsqueeze(2).broadcast_to([128, jl, 56])
                t3 = p2s.tile([128, 28, 56], F32, tag="t1")
                nc.vector.tensor_tensor(t3[:, 0:jl, :], everow, arow, op=ALU.mult)
                t4 = p2s.tile([128, 28, 56], F32, tag="t2")
                nc.vector.tensor_tensor(t4[:, 0:jl, :], oddrow, brow, op=ALU.mult)
                nc.vector.tensor_tensor(
                    x2rv[:, jc:jc + jl, :], t3[:, 0:jl, :], t4[:, 0:jl, :], op=ALU.add
                )
            nc.vector.tensor_copy(x2rv[:, 55:56, :], x2wv[:, 111:112, :])

            # ---- P3: f8_3 = relu(f83T.T @ x2r) -> f_b[0:64] ----
            with tc.tile_pool(name="p3p", bufs=4, space=bass.MemorySpace.PSUM) as p3p:
                for no, nl in NCH:
                    fp = p3p.tile([64, 512], F32, tag="f3psum")
                    nc.tensor.matmul(
                        fp[:, 0:nl], f83T[:], x2r[:, no:no + nl], start=True, stop=True
                    )
                    nc.scalar.activation(f_b[0:64, no:no + nl], fp[:, 0:nl], AF.Relu)

        # ================= P4: x -> x_s -> f_b[64:67] =================
        with tc.tile_pool(name="p4s", bufs=2) as p4s, \
             tc.tile_pool(name="p4sb", bufs=1) as p4sb, \
             tc.tile_pool(name="p4p", bufs=1, space=bass.MemorySpace.PSUM) as p4p:
            xh = p4sb.tile([56, 3, 448], BF16, tag="xh")
            xps = [
                p4p.tile([56, 448], F32, tag=f"xhp{c}", name=f"xhp{c}")
                for c in range(3)
            ]
            xdr = d_x.ap().rearrange("c h w -> h c w")
            for hc in range(4):
                st = p4s.tile([112, 3, 448], BF16, tag="xst")
                nc.sync.dma_start(st[:], xdr[112 * hc:112 * (hc + 1), :, :])
                for c in range(3):
                    nc.tensor.matmul(
                        xps[c][:], rh[:, hc, :], st[:, c, :],
                        start=(hc == 0), stop=(hc == 3),
                    )
            for c in range(3):
                nc.vector.tensor_copy(xh[:, c, :], xps[c][:])

            xhT = p4sb.tile([112, 12, 56], BF16, tag="xhT")
            idb = p4sb.tile([128, 128], BF16, tag="idb")
            nc.vector.tensor_copy(idb[:], ident[:])
            for c in range(3):
                for wc in range(4):
                    tp = p4p.tile([112, 56], BF16, tag="xtp", bufs=2)
                    nc.tensor.transpose(
                        tp[:], xh[:, c, 112 * wc:112 * (wc + 1)], idb[0:56, 0:56]
                    )
                    nc.vector.tensor_copy(xhT[:, c * 4 + wc, :], tp[:])
            for c in range(3):
                wp = p4p.tile([56, 56], F32, tag="xwp", bufs=2)
                for wc in range(4):
                    nc.tensor.matmul(
                        wp[:], xhT[:, c * 4 + wc, :], rw[:, wc, :],
                        start=(wc == 0), stop=(wc == 3),
                    )
                ws = p4s.tile([56, 56], F32, tag="xws")
                nc.vector.tensor_copy(ws[:], wp[:])
                nc.sync.dma_start(f_b[64 + c:65 + c, :], ws[:])

        # ================= P5: f8_4 = relu(f84T.T @ deep3) -> f_a =================
        with tc.tile_pool(name="p5s", bufs=4) as p5s, \
             tc.tile_pool(name="p5p", bufs=4, space=bass.MemorySpace.PSUM) as p5p:
            DCH = [(0, 128), (128, 128), (256, 64)]
            for no, nl in NCH:
                fp = p5p.tile([128, 512], F32, tag="f4psum")
                for ci, (co, cl) in enumerate(DCH):
                    st = p5s.tile([128, 512], F32, tag="d3st")
                    nc.sync.dma_start(
                        st[0:cl, 0:nl], d_deep3.ap()[co:co + cl, no:no + nl]
                    )
                    w = (f84T_0, f84T_1, f84T_2)[ci]
                    nc.tensor.matmul(
                        fp[:, 0:nl], w[:], st[0:cl, 0:nl],
                        start=(ci == 0), stop=(ci == 2),
                    )
                nc.scalar.activation(f_a[:, no:no + nl], fp[:, 0:nl], AF.Relu)

        # ================= P6: q, k =================
        with tc.tile_pool(name="p6p", bufs=4, space=bass.MemorySpace.PSUM) as p6p:
            MCH = [(qA, 0, 128), (qB, 128, 64), (kA, 192, 128), (kB, 320, 64)]
            for dst, mo, ml in MCH:
                for no, nl in NCH:
                    qp = p6p.tile([128, 512], F32, tag="qkpsum")
                    nc.tensor.matmul(
                        qp[0:ml, 0:nl], qkA[:, mo:mo + ml], f_a[:, no:no + nl],
                        start=True, stop=False,
                    )
                    nc.tensor.matmul(
                        qp[0:ml, 0:nl], qkB[:, mo:mo + ml], f_b[:, no:no + nl],
                        start=False, stop=True,
                    )
                    nc.vector.tensor_copy(dst[0:ml, no:no + nl], qp[0:ml, 0:nl])

        # ================= P7: attention =================
        with tc.tile_pool(name="p7e", bufs=6) as p7e, \
             tc.tile_pool(name="p7r", bufs=2) as p7r, \
             tc.tile_pool(name="p7s", bufs=2, space=bass.MemorySpace.PSUM) as p7s, \
             tc.tile_pool(name="p7o", bufs=2, space=bass.MemorySpace.PSUM) as p7o:
            # k-superblocks: 3 x 1024 + 1 x 64
            KSUP = [(0, 1024), (1024, 1024), (2048, 1024), (3072, 64)]
            for ko, kl in KSUP:
                nkb = (kl + 511) // 512
                pout = p7o.tile([5, 1024], F32, tag="pout")
                for bi, (ho, hl) in enumerate(HBLK):
                    sp = p7s.tile([128, 1024], F32, tag="spsum")
                    for kb in range(nkb):
                        kbl = min(512, kl - kb * 512)
                        nc.tensor.matmul(
                            sp[0:hl, kb * 512:kb * 512 + kbl], qA[:, ho:ho + hl],
                            kA[:, ko + kb * 512:ko + kb * 512 + kbl],
                            start=True, stop=False,
                        )
                    for kb in range(nkb):
                        kbl = min(512, kl - kb * 512)
                        nc.tensor.matmul(
                            sp[0:hl, kb * 512:kb * 512 + kbl], qB[:, ho:ho + hl],
                            kB[:, ko + kb * 512:ko + kb * 512 + kbl],
                            start=False, stop=True,
                        )
                    et = p7e.tile([128, 1024], BF16, tag="exptile")
                    nc.scalar.activation(et[0:hl, 0:kl], sp[0:hl, 0:kl], AF.Exp)
                    for kb in range(nkb):
                        kbl = min(512, kl - kb * 512)
                        nc.tensor.matmul(
                            pout[:, kb * 512:kb * 512 + kbl],
                            camT5[0:hl, bi * 5:bi * 5 + 5],
                            et[0:hl, kb * 512:kb * 512 + kbl],
                            start=(bi == 0), stop=(bi == 24),
                        )
                ot5 = p7r.tile([5, 1024], F32, tag="ot5")
                nc.vector.tensor_copy(ot5[:, 0:kl], pout[:, 0:kl])
                den = p7r.tile([1, 1024], F32, tag="den")
                nc.sync.dma_start(den[0:1, 0:kl], ot5[4:5, 0:kl])
                rcp = p7r.tile([1, 1024], F32, tag="rcp")
                nc.vector.reciprocal(rcp[0:1, 0:kl], den[0:1, 0:kl])
                rb = p7r.tile([4, 1024], F32, tag="rb")
                nc.gpsimd.partition_broadcast(rb[:, 0:kl], rcp[0:1, 0:kl])
                nc.gpsimd.tensor_tensor(
                    out_sb[:, ko:ko + kl], ot5[0:4, 0:kl], rb[:, 0:kl], op=ALU.mult
                )

        nc.sync.dma_start(d_out.ap(), out_sb[:])

    nc.compile()
    return nc


def _get_program():
    if "nc" not in _CACHE:
        _CACHE["nc"] = _build_program()
    return _CACHE["nc"]


def _host_prep(inputs: dict) -> list[dict]:
    x = np.ascontiguousarray(np.asarray(inputs["x"], np.float32))
    x2 = np.ascontiguousarray(np.asarray(inputs["x2"], np.float32))
    deep3 = np.ascontiguousarray(np.asarray(inputs["deep3"], np.float32))
    _4 = np.ascontiguousarray(np.asarray(inputs["_4"], np.float32))
    fc8_w = np.asarray(inputs["fc8_w"], np.float32)
    f83_w = np.asarray(inputs["f83_w"], np.float32)
    f84_w = np.asarray(inputs["f84_w"], np.float32)
    f91_w = np.asarray(inputs["f91_w"], np.float32)
    f92_w = np.asarray(inputs["f92_w"], np.float32)

    n = x.shape[0]
    fc8T = np.ascontiguousarray(fc8_w.T)  # [512, 4]
    f83T = np.ascontiguousarray(f83_w.T)  # [128, 64]
    f84T = np.ascontiguousarray(f84_w.T)  # [320, 128]
    # f channel permutation: [f8_4 (128), f8_3 (64), x_s (3)]
    perm = np.concatenate([np.arange(67, 195), np.arange(3, 67), np.arange(3)])
    wqk = np.concatenate([f91_w, f92_w], axis=0)[:, perm]  # [384, 195]
    wqkT = np.ascontiguousarray(wqk.T)  # [195, 384]
    qkA = np.ascontiguousarray(wqkT[0:128])
    qkB = np.ascontiguousarray(wqkT[128:195])
    a112, b112 = _resize_coeffs_112()
    import ml_dtypes

    BFNP = ml_dtypes.bfloat16
    a112 = np.ascontiguousarray(np.broadcast_to(a112, (128, 56)))
    b112 = np.ascontiguousarray(np.broadcast_to(b112, (128, 56)))
    rh448 = _resize_mat(448, 56).astype(BFNP)
    rw448 = rh448  # same matrix for H and W (448x448 -> 56x56)
    x = x.astype(BFNP)

    shared = {
        "fc8T": fc8T, "f83T": f83T, "f84T": f84T, "qkA": qkA, "qkB": qkB,
        "a112": a112, "b112": b112, "rh448": rh448, "rw448": rw448,
    }
    in_maps = []
    for i in range(n):
        m = dict(shared)
        m["x4"] = _4[i].reshape(512, HW)
        m["deep3"] = deep3[i].reshape(320, HW)
        m["x2"] = x2[i].reshape(128, 112 * 112)
        m["x"] = x[i]
        in_maps.append(m)
    return in_maps


def _install_ntff_hook() -> bool:
    """Register the NTFF profile hook that the agent image's antenv lacks."""
    try:
        import types

        import antenv

        if "antenv.axon_hooks" not in sys.modules:
            mod = types.ModuleType("antenv.axon_hooks")
            store = {"h": None}
            mod.set_axon_ntff_profile_hook = lambda h: store.update(h=h)
            mod.get_axon_ntff_profile_hook = lambda: store["h"]
            sys.modules["antenv.axon_hooks"] = mod
            antenv.axon_hooks = mod
            from trn_agent_boot.trn_boot import _ntff_profile_via_ctypes

            hook = _ntff_profile_via_ctypes("/opt/axon/libaxon_pjrt.so")
            if hook is None:
                return False
            mod.set_axon_ntff_profile_hook(hook)
        return sys.modules["antenv.axon_hooks"].get_axon_ntff_profile_hook() is not None
    except Exception as e:  # profiling is best-effort
        print(f"ntff hook install failed: {e}", file=sys.stderr)
        return False


def kernel(**inputs) -> np.ndarray:
    nc = _get_program()
    in_maps = _host_prep(inputs)
    trace = bool(int(os.environ.get("KERNEL_PROFILE", "0")))
    if trace:
        trace = _install_ntff_hook()
    res = run_bass_kernel_spmd(nc, in_maps, core_ids=list(range(N_CORES)),
                               trace=trace)
    _CACHE["last_result"] = res
    out = np.stack([r["out"] for r in res.results]).reshape(8, 4, 56, 56)
    return out.astype(np.float32)
